# revision 24
# baseline (speedup 1.0000x reference)
"""Trainium2 Bass kernel for nn_AttentionLayer (attention pooling).

Reference math (per batch row b):
    u   = tanh(x[b] @ W + b_vec)        # [T, M]
    s   = u @ us                        # [T]
    a   = softmax(s) * mask / sum       # [T]  (mask is all ones per spec)
    out = a @ x[b]                      # [D]

Strategy: data-parallel over batch, B=32 rows -> 4 rows per NeuronCore on
8 cores.  The kernel is HBM-stream-bound (~85-100us to stream x at the
per-core share of HBM bandwidth), so every engine is kept under the
stream budget and the last row's chain is arranged so almost nothing
trails the final DMA:
  - x is uploaded host-transposed per row as xh[q, p, c, tq] =
    x[t, 128c + p] (t = 512q + tq): each quarter DMA reads 16KB
    contiguous per partition and lands as x^T tiles [p=d, c, t] directly
    usable as the d-contracted matmul rhs;
  - the f32->bf16 cast rides the DMA itself (SWDGE dtype-cast);
  - u^T = tanh(W^T x^T + b) in PSUM, tanh fused on ScalarE;
  - scores: us is replicated into all 128 PE columns, so one N=512
    matmul per quarter yields broadcast scores s[128, 512] in PSUM; exp
    on ScalarE writes broadcast e-rows plus the row-sum partials Z
    (accum_out);
  - pooling out[d] = sum_t e[t] x^T[d, t]:
      rows 0..2: chunks 0..5 on the DVE as one fused
        scalar_tensor_tensor (mult+mult, accum_out=sum) granule per row
        (coarse granules amortize the DVE accumulator-drain turnaround),
        chunks 6..7 on the PE as e^T-column matmuls into a column PSUM
        (which also keeps the PE's HAM activity window alive);
      row 3 (the last): entirely on the PE as two co-running [1, 512]
        row-matmuls per t-tile (lhsT = e^T column), since the DVE's
        serial chain would trail the stream end;
  - one-slot software pipeline: each piece's exp-dependent PE block
    (e^T transpose + e-col copy + c2 matmuls) is deferred into the next
    piece's slot so the PE never stalls waiting for exp; row finishes
    ride the same deferral;
  - y rows 0..2 are stored [p, c] and unshuffled on the host; the last
    row is produced as a plain [1, D] row (y3).
"""
import numpy as np

import concourse.bacc as bacc
import concourse.mybir as mybir
from concourse.tile import TileContext
from concourse.masks import make_identity
from concourse.bass_utils import run_bass_kernel_spmd

F32 = mybir.dt.float32
BF16 = mybir.dt.bfloat16

B, T, D, M = 32, 2048, 1024, 128
NCORES = 8
B_SH = B // NCORES   # 4 batch rows per core
P = 128
NCD = D // P         # 8 d-chunks
NQ = 4               # t-quarters per row
TQ = T // NQ         # 512 t per quarter
NTT = TQ // P        # 4 t-tiles per quarter
LAST = B_SH - 1
WARMUP = 40
NPE = 2              # chunks pooled on PE for rows 0..2
NDVE = NCD - NPE


def _pieces_of(r, q):
    # last-row tail: final quarter split so the post-stream chain is short
    if r == LAST and q == NQ - 1:
        return [(0, 384), (384, 128)]
    return [(0, TQ)]


def _dma_pieces_of(r, q):
    # first DMA split small so descriptor-gen latency doesn't delay the
    # stream start; elsewhere 2MB quarters
    if r == 0 and q == 0:
        return [(0, 128), (128, 128), (256, 256)]
    return _pieces_of(r, q)


def _build_nc():
    nc = bacc.Bacc("TRN2", target_bir_lowering=False, debug=False,
                   num_devices=NCORES)
    # x host-rearranged: xh[r, q, p, c, tq] = x[r, 512q+tq, 128c+p]
    x = nc.declare_dram_parameter("x", [B_SH, NQ, P, NCD, TQ], F32,
                                  isOutput=False)
    # W host-rearranged to lhsT layout: W_r[p, c, m] = W[128c+p, m]
    W = nc.declare_dram_parameter("W", [P, NCD, M], F32, isOutput=False)
    b = nc.declare_dram_parameter("b", [1, M], F32, isOutput=False)
    us = nc.declare_dram_parameter("us", [1, M], F32, isOutput=False)
    # y[r, p, c] = out[r, 128c+p] for rows 0..2; y3 = out[3] as a row
    y = nc.declare_dram_parameter("y", [B_SH - 1, P, NCD], F32, isOutput=True)
    y3 = nc.declare_dram_parameter("y3", [1, D], F32, isOutput=True)

    with TileContext(nc) as tc:
        with (
            tc.tile_pool(name="singles", bufs=1) as singles,
            tc.tile_pool(name="xb", bufs=3) as xb_pool,
            tc.tile_pool(name="esb", bufs=3) as e_pool,
            tc.tile_pool(name="usb", bufs=3) as u_pool,
            tc.tile_pool(name="xn", bufs=3) as xn_pool,
            tc.tile_pool(name="xn8", bufs=8) as xn8_pool,
            tc.tile_pool(name="ec", bufs=3) as ec_pool,
            tc.tile_pool(name="rowacc", bufs=8) as acc_pool,
            tc.tile_pool(name="scr", bufs=2) as scr_pool,
            tc.tile_pool(name="fin", bufs=2) as fin_pool,
            tc.tile_pool(name="up_ps", bufs=2, space="PSUM") as u_psum,
            tc.tile_pool(name="sb_ps", bufs=1, space="PSUM") as s_psum,
            tc.tile_pool(name="tp_ps", bufs=3, space="PSUM") as tp_psum,
            tc.tile_pool(name="oc_ps", bufs=1, space="PSUM") as oc_psum,
            tc.tile_pool(name="or_ps", bufs=1, space="PSUM") as or_psum,
        ):
            # constants on the sync HWDGE queue (separate from the x
            # stream's SWDGE queue)
            w_f32 = singles.tile([P, NCD, M], F32)
            nc.sync.dma_start(out=w_f32, in_=W[:, :, :])
            b_row = singles.tile([1, M], F32)
            nc.sync.dma_start(out=b_row, in_=b[:, :])
            us_row = singles.tile([1, M], F32)
            nc.sync.dma_start(out=us_row, in_=us[:, :])

            # x stream: SWDGE cast-DMA straight into bf16 row buffers
            xb_tiles = {}

            def emit_row_dmas(r):
                xb_r = xb_pool.tile([P, NQ, NCD, TQ], BF16, tag="xb",
                                    name=f"xb_{r}")
                xb_tiles[r] = xb_r
                for q in range(NQ):
                    for (t0, tw) in _dma_pieces_of(r, q):
                        nc.gpsimd.dma_start(
                            out=xb_r[:, q, :, t0:t0 + tw],
                            in_=x[r, q][:, :, t0:t0 + tw],
                        )

            emit_row_dmas(0)

            # init constants on DVE
            one_f32 = singles.tile([1, 1], F32)
            nc.vector.memset(one_f32, 1.0)
            ones_bf = singles.tile([P, P], BF16)
            nc.vector.memset(ones_bf, 1.0)
            ident = singles.tile([P, P], BF16)
            make_identity(nc, ident)
            w_bf = singles.tile([P, NCD, M], BF16)
            nc.vector.tensor_copy(out=w_bf, in_=w_f32)

            emit_row_dmas(1)
            emit_row_dmas(2)

            # PE warm-up while the first quarters stream in (into the
            # up-tag ring so no extra PSUM bank is spent)
            warm = u_psum.tile([P, TQ], F32, tag="up", name="warm")
            for i in range(WARMUP):
                nc.tensor.matmul(warm[:, 0:P], ones_bf, ones_bf,
                                 start=True, stop=True)

            # b/us -> per-partition layout via K=1 matmuls, into the sb
            # ring's first slot (saves a PSUM bank)
            bc = s_psum.tile([P, TQ], F32, tag="sb", name="bc")
            nc.tensor.matmul(bc[:, 0:1], b_row, one_f32, start=True, stop=True)
            nc.tensor.matmul(bc[:, 1:2], us_row, one_f32, start=True, stop=True)
            b_sb = singles.tile([P, 1], F32)
            nc.vector.tensor_copy(out=b_sb, in_=bc[:, 0:1])
            us_sc = singles.tile([P, 1], F32)
            nc.vector.tensor_copy(out=us_sc, in_=bc[:, 1:2])
            # us replicated into 128 identical PE columns
            us_bc = singles.tile([P, P], BF16)
            nc.vector.tensor_scalar_mul(us_bc, ones_bf, us_sc)

            # last row's pooled rows: [1,512] at partition 0 (d 0..511) and
            # partition 64 (d 512..1023), both in one PSUM bank
            oprow = or_psum.tile([P, TQ], F32, tag="or")

            # one-slot software pipeline (see module docstring)
            deferred = [None]
            pending_fin = [None]

            def drain():
                if deferred[0] is not None:
                    f, deferred[0] = deferred[0], None
                    f()
                if pending_fin[0] is not None:
                    f, pending_fin[0] = pending_fin[0], None
                    f()

            for r in range(B_SH):
                xb_r = xb_tiles[r]
                e_sb = e_pool.tile([P, NQ, TQ], BF16, tag="e", name=f"e_{r}")
                rs = acc_pool.tile([P, 8], F32, tag="rs", name=f"rs_{r}")
                n_rs = 0
                last_row = r == LAST

                if r + 3 < B_SH:
                    emit_row_dmas(r + 3)

                if not last_row:
                    oc = oc_psum.tile([P, NPE], F32, tag="oc",
                                      name=f"oc_{r}")
                    oc_i = [0]
                    n_oc = NPE * NQ * NTT
                acc = [None]
                c2_i = [0]
                n_c2 = 2 * NQ * NTT   # row 3 only

                for q in range(NQ):
                    ecq = ec_pool.tile([P, NTT], BF16, tag="ec",
                                       name=f"ec_{r}_{q}")
                    if not last_row:
                        xn67 = xn_pool.tile([P, NPE, NTT, P], BF16,
                                            tag="xn67")
                    for (t0, tw) in _pieces_of(r, q):
                        up = u_psum.tile([P, TQ], F32, tag="up")
                        for c in range(NCD):
                            nc.tensor.matmul(
                                up[:, t0:t0 + tw],
                                w_bf[:, c, :],
                                xb_r[:, q, c, t0:t0 + tw],
                                start=(c == 0), stop=(c == NCD - 1),
                            )
                        # x^T -> natural transposes for the PE-pooled chunks;
                        # only xb-dependent, so they run under the stream
                        xn_piece = {}
                        if last_row:
                            for j in range(t0 // P, (t0 + tw) // P):
                                tpa = tp_psum.tile([P, 4 * P], BF16, tag="tp")
                                tpb = tp_psum.tile([P, 4 * P], BF16, tag="tp")
                                for c in range(4):
                                    nc.tensor.transpose(
                                        tpa[:, c * P:(c + 1) * P],
                                        xb_r[:, q, c, j * P:(j + 1) * P],
                                        ident)
                                    nc.tensor.transpose(
                                        tpb[:, c * P:(c + 1) * P],
                                        xb_r[:, q, 4 + c, j * P:(j + 1) * P],
                                        ident)
                                xn = xn8_pool.tile([P, NCD, P], BF16,
                                                   tag="xn8")
                                xn_piece[j] = xn
                                nc.vector.tensor_copy(
                                    out=xn[:, 0:4, :],
                                    in_=tpa.rearrange("p (c t) -> p c t",
                                                      c=4))
                                nc.scalar.copy(
                                    out=xn[:, 4:8, :],
                                    in_=tpb.rearrange("p (c t) -> p c t",
                                                      c=4))
                        elif t0 == 0:
                            tpx = tp_psum.tile([P, 4 * P], BF16, tag="tp")
                            tpy = tp_psum.tile([P, 4 * P], BF16, tag="tp")
                            for j in range(NTT):
                                nc.tensor.transpose(
                                    tpx[:, j * P:(j + 1) * P],
                                    xb_r[:, q, NDVE, j * P:(j + 1) * P],
                                    ident)
                                nc.tensor.transpose(
                                    tpy[:, j * P:(j + 1) * P],
                                    xb_r[:, q, NDVE + 1, j * P:(j + 1) * P],
                                    ident)
                            nc.scalar.copy(
                                out=xn67[:, 0, :, :],
                                in_=tpx.rearrange("p (j t) -> p j t", j=NTT))
                            nc.scalar.copy(
                                out=xn67[:, 1, :, :],
                                in_=tpy.rearrange("p (j t) -> p j t", j=NTT))
                        drain()
                        u_sb = u_pool.tile([P, TQ], BF16, tag="u")
                        nc.scalar.activation(
                            out=u_sb[:, t0:t0 + tw], in_=up[:, t0:t0 + tw],
                            func=mybir.ActivationFunctionType.Tanh,
                            bias=b_sb, scale=1.0,
                        )
                        sb = s_psum.tile([P, TQ], F32, tag="sb")
                        nc.tensor.matmul(
                            sb[:, t0:t0 + tw], us_bc, u_sb[:, t0:t0 + tw],
                            start=True, stop=True,
                        )
                        nc.scalar.activation(
                            out=e_sb[:, q, t0:t0 + tw], in_=sb[:, t0:t0 + tw],
                            func=mybir.ActivationFunctionType.Exp,
                            accum_out=rs[:, n_rs:n_rs + 1],
                        )
                        n_rs += 1

                        if last_row:
                            def pool_pe(q=q, t0=t0, tw=tw, e_sb=e_sb,
                                        ecq=ecq, xn_piece=xn_piece,
                                        c2_i=c2_i):
                                # e^T columns via PE transpose of the
                                # broadcast e-rows, then two co-running
                                # [1,512] row-matmuls per t-tile
                                j0, j1 = t0 // P, (t0 + tw) // P
                                tpe = tp_psum.tile([P, 4 * P], BF16,
                                                   tag="tp")
                                for j in range(j0, j1):
                                    nc.tensor.transpose(
                                        tpe[:, (j - j0) * P:(j - j0 + 1) * P],
                                        e_sb[:, q, j * P:(j + 1) * P], ident)
                                nc.scalar.copy(
                                    out=ecq[:, j0:j1],
                                    in_=tpe.rearrange(
                                        "p (j t) -> p j t",
                                        j=4)[:, 0:j1 - j0, 0])
                                for j in range(j0, j1):
                                    xn = xn_piece[j]
                                    nc.tensor.matmul(
                                        oprow[0:1, :], ecq[:, j:j + 1],
                                        xn[:, 0:4, :],
                                        start=(c2_i[0] == 0), stop=False)
                                    c2_i[0] += 1
                                    nc.tensor.matmul(
                                        oprow[64:65, :], ecq[:, j:j + 1],
                                        xn[:, 4:8, :],
                                        start=False,
                                        stop=(c2_i[0] == n_c2 - 1),
                                        tile_position=(0, 64))
                                    c2_i[0] += 1
                        else:
                            def pool_pe(q=q, t0=t0, tw=tw, e_sb=e_sb,
                                        ecq=ecq, xn67=xn67, oc=oc,
                                        oc_i=oc_i, n_oc=n_oc):
                                # e^T columns + c2 col-matmuls for chunks
                                # 6..7 (single start/stop for the per-row
                                # group: start clears the whole bank)
                                j0, j1 = t0 // P, (t0 + tw) // P
                                tpe = tp_psum.tile([P, 4 * P], BF16,
                                                   tag="tp")
                                for j in range(j0, j1):
                                    nc.tensor.transpose(
                                        tpe[:, (j - j0) * P:(j - j0 + 1) * P],
                                        e_sb[:, q, j * P:(j + 1) * P], ident)
                                nc.scalar.copy(
                                    out=ecq[:, j0:j1],
                                    in_=tpe.rearrange(
                                        "p (j t) -> p j t",
                                        j=4)[:, 0:j1 - j0, 0])
                                for j in range(j0, j1):
                                    for ci in range(NPE):
                                        nc.tensor.matmul(
                                            oc[:, ci:ci + 1],
                                            xn67[:, ci, j, :],
                                            ecq[:, j:j + 1],
                                            start=(oc_i[0] == 0),
                                            stop=(oc_i[0] == n_oc - 1))
                                        oc_i[0] += 1

                        deferred[0] = pool_pe
                    # rows 0..2: one coarse DVE granule per row, emitted at
                    # row end (amortizes the DVE accumulator turnaround)
                    if not last_row and q == NQ - 1:
                        dst = acc_pool.tile([P, NDVE], F32, tag="acc",
                                            name=f"acc_{r}")
                        acc[0] = dst
                        scr = scr_pool.tile([P, NQ, TQ], BF16, tag="scr",
                                            name=f"scr_{r}")
                        for c in range(NDVE):
                            nc.vector.scalar_tensor_tensor(
                                out=scr,
                                in0=xb_r[:, :, c, :],
                                scalar=1.0,
                                in1=e_sb[:, :, :],
                                op0=mybir.AluOpType.mult,
                                op1=mybir.AluOpType.mult,
                                accum_out=dst[:, c:c + 1],
                            )

                if not last_row:
                    def finish(r=r, rs=rs, n_rs=n_rs, acc=acc, oc=oc):
                        zr = fin_pool.tile([P, 1], F32, tag="z")
                        nc.vector.tensor_reduce(
                            out=zr, in_=rs[:, 0:n_rs],
                            axis=mybir.AxisListType.X, op=mybir.AluOpType.add)
                        inv = fin_pool.tile([P, 1], F32, tag="inv")
                        nc.vector.reciprocal(out=inv, in_=zr)
                        y_sb = fin_pool.tile([P, NCD], F32, tag="y")
                        nc.vector.tensor_scalar_mul(y_sb[:, 0:NDVE], acc[0],
                                                    inv)
                        nc.vector.tensor_scalar_mul(y_sb[:, NDVE:NCD], oc,
                                                    inv)
                        nc.sync.dma_start(out=y[r], in_=y_sb)
                else:
                    def finish(r=r, rs=rs, n_rs=n_rs):
                        zr = fin_pool.tile([P, 1], F32, tag="z")
                        nc.vector.tensor_reduce(
                            out=zr, in_=rs[:, 0:n_rs],
                            axis=mybir.AxisListType.X, op=mybir.AluOpType.add)
                        inv = fin_pool.tile([P, 1], F32, tag="inv")
                        nc.vector.reciprocal(out=inv, in_=zr)
                        y3_sb = fin_pool.tile([1, D], F32, tag="y3")
                        # opa on partition 0, opb on partition 64; split the
                        # scales across DVE and ACT so they run concurrently
                        nc.vector.tensor_scalar_mul(
                            y3_sb[:, 0:TQ], oprow[0:1, :], inv[0:1, :])
                        nc.scalar.activation(
                            out=y3_sb[:, TQ:2 * TQ], in_=oprow[64:65, :],
                            func=mybir.ActivationFunctionType.Copy,
                            scale=inv[64:65, :])
                        nc.sync.dma_start(out=y3[0:1, :], in_=y3_sb)

                pending_fin[0] = finish

            # flush the pipeline: last piece's pooling block + last finish
            drain()

    nc.compile()
    return nc


_NC_CACHE = []


def _numpy_reference(x, W, b, us, mask):
    m = mask.astype(x.dtype)
    u = np.tanh(np.einsum('btd,dm->btm', x, W) + b)
    utu = np.einsum('btm,mo->bto', u, us)[..., 0]
    e = np.exp(utu - utu.max(axis=-1, keepdims=True))
    e = m * e
    a = e / e.sum(axis=-1, keepdims=True)
    return np.einsum('bt,btd->bd', a, x).astype(np.float32)


def make_in_maps(x, W, b, us):
    """Per-core input dicts; x/W host-rearranged (layout only, dtypes kept)."""
    x = np.ascontiguousarray(np.asarray(x, dtype=np.float32))
    W = np.ascontiguousarray(np.asarray(W, dtype=np.float32))
    b = np.ascontiguousarray(np.asarray(b, dtype=np.float32))
    us = np.ascontiguousarray(np.asarray(us, dtype=np.float32))
    W_r = np.ascontiguousarray(W.reshape(NCD, P, M).transpose(1, 0, 2))
    b_r = np.ascontiguousarray(b.reshape(1, M))
    us_r = np.ascontiguousarray(us.reshape(M, 1).T)
    maps = []
    for i in range(NCORES):
        xc = x[i * B_SH:(i + 1) * B_SH]                # [B_SH, T, D]
        xh = xc.reshape(B_SH, NQ, TQ, NCD, P).transpose(0, 1, 4, 3, 2)
        maps.append({
            "x": np.ascontiguousarray(xh),
            "W": W_r, "b": b_r, "us": us_r,
        })
    return maps


def gather_outputs(res):
    outs = []
    for i in range(NCORES):
        yr = res.results[i]["y"]               # [B_SH-1, P, NCD]
        rows = yr.transpose(0, 2, 1).reshape(B_SH - 1, D)
        last = res.results[i]["y3"].reshape(1, D)
        outs.append(np.concatenate([rows, last], axis=0))
    return np.ascontiguousarray(np.concatenate(outs, axis=0))


def kernel(x, W, b, us, mask):
    x = np.ascontiguousarray(np.asarray(x, dtype=np.float32))
    W = np.ascontiguousarray(np.asarray(W, dtype=np.float32))
    b = np.ascontiguousarray(np.asarray(b, dtype=np.float32))
    us = np.ascontiguousarray(np.asarray(us, dtype=np.float32))
    mask = np.asarray(mask)

    if not bool(mask.all()):
        # spec guarantees an all-ones mask; exact numpy fallback otherwise
        return _numpy_reference(x, W, b, us, mask)

    if not _NC_CACHE:
        _NC_CACHE.append(_build_nc())
    nc = _NC_CACHE[0]

    in_maps = make_in_maps(x, W, b, us)
    res = run_bass_kernel_spmd(nc, in_maps, core_ids=list(range(NCORES)),
                               trace=False)
    return gather_outputs(res)
